# revision 19
# baseline (speedup 1.0000x reference)
"""Trainium2 Bass kernel for channel-wise ("transposed") attention.

Reference computation (per batch b, X = x_in[b] reshaped [N=16384, C=256]):
    Q = X Wq ; K = X Wk ; V = X Wv            (columns l2-normalized over tokens for Q,K)
    attn[h,i,j] = softmax_j( khat_i . qhat_j * rescale[h] )   (32x32 per head)
    out = (A_bd @ V^T)^T Wp + bp

Algebraic reduction (validated vs reference):
    S    = X^T X                      [256,256]   (only pass-1 reduction needed)
    P1   = S Wq ; P2 = S Wk
    G    = Wk^T P1                    (raw cross-gram K^T Q)
    nq2  = colsum(Wq*rexp^-2 . P1) ; nk2 = diag(Wk^T P2)
    L    = G * rk[i] * (rq*rescale)[j] ;  A = blockdiag-softmax_j(exp(L))
    Wbig = Wv @ (A_bd^T Wp)           [256,256]
    out  = X @ Wbig + bp

Numerics: whole data path in fp16 (not bf16): fp16's 10-bit mantissa keeps the
end-to-end rel err ~7e-4 (vs ~2e-2 with bf16) at the same 1 cyc/row matmul
rate.  All accumulation is f32 PSUM.

Schedule (per core = one batch, data parallel, no collectives):
  pass 1   X streams in as fp16 via 8 big casting DMAs on the Pool/SWDGE
           queue (the ~1us SWDGE fixed cost per DMA instruction makes many
           small DMAs Pool-bound).  Tokens are blocked 16-per-partition so
           each DMA needs only 128 descriptors.  Weights load as f32 on the
           SP/HWDGE queue (casting DMAs are Pool-only) and are downcast to
           fp16 by cheap 2x_2p DVE/ACT copies.  PE: symmetric S accumulation
           (S00|S01 fused 256-wide + S11, 384 cyc/tile) with X-tile
           transposes (xT) filling the DMA slack.
  phase B  tiny 256x256 chains -> Wbig, all fp16 matmuls.  The head-block
           mask is folded into the G PSUM as rank-1+rank-4 matmuls with -B
           outside blocks, so the softmax row-sum comes free from the ACT
           Exp accumulator.  rsqrt via exp(-0.5 ln x) (act set 6).
           Leftover transpose quads fill PE stalls.
  pass 2   out^T = Wbig^T xT + bp computed transposed [C, N]: bias is a
           per-partition [P,1] operand fused into the PSUM evictions on both
           DVE and ACT.  Output quads cover contiguous true-token ranges
           (the eviction APs undo the blocked-token permutation), stream out
           as 32 pipelined fp16 DMAs; host transposes/casts back.
"""

import sys

if "/opt/trn_rl_repo" not in sys.path:
    sys.path.insert(0, "/opt/trn_rl_repo")

from contextlib import ExitStack

import numpy as np

import concourse.bass as bass
import concourse.tile as tile
from concourse import bacc, mybir
from concourse import bass_utils
from concourse.bass import ds, ts
from concourse.bass_interp import get_hw_module
from concourse.masks import make_identity

F32 = mybir.dt.float32
F32R = mybir.dt.float32r
F16 = mybir.dt.float16
F8 = mybir.dt.float8e4
ALU = mybir.AluOpType
ACTF = mybir.ActivationFunctionType
PSUM = bass.MemorySpace.PSUM

N_CORES = 8
B, H, W, C = 8, 128, 128, 256
HEADS, DH = 8, 32
N = H * W            # 16384 tokens per batch
P = 128              # partitions / token tile
NT = N // P          # 128 token tiles
GT = 16              # token tiles per DMA group (2048 tokens)
NG = NT // GT        # 8 groups
NCHUNK = C // P      # 2 channel chunks
QT = 4               # token tiles per transpose/output quad
NQ = NT // QT        # 32 quads

# act_func_sets index of natural_log_exp_and_others: {ln, exp, copy, identity}
ACT_SET_LN_EXP = 6

# Block-diag mask magnitude: logits get -MROW*MCOL outside head blocks before
# the rq/rk normalization scales (~6e-5 combined), leaving ~-32 in the exp.
MROW = 1024.0
MCOL = 512.0


def _build_kernel(nc: bacc.Bacc):
    x_dram = nc.dram_tensor("x_in", [N, C], F32, kind="ExternalInput").ap()
    wq_dram = nc.dram_tensor("Wq", [C, C], F32, kind="ExternalInput").ap()
    wk_dram = nc.dram_tensor("Wk", [C, C], F32, kind="ExternalInput").ap()
    wv_dram = nc.dram_tensor("Wv", [C, C], F32, kind="ExternalInput").ap()
    resc_dram = nc.dram_tensor("rescale", [HEADS, 1, 1], F32, kind="ExternalInput").ap()
    wp_dram = nc.dram_tensor("Wp", [C, C], F32, kind="ExternalInput").ap()
    bp_dram = nc.dram_tensor("bp", [C], F32, kind="ExternalInput").ap()
    # output is stored transposed [C, N] fp16; host casts + transposes back
    out_dram = nc.dram_tensor("out", [C, N], F16, kind="ExternalOutput").ap()
    outT_v = out_dram.rearrange("(k p) n -> p k n", p=P)

    with tile.TileContext(nc) as tc, ExitStack() as top:
        consts = top.enter_context(tc.tile_pool(name="consts", bufs=1))
        xt_pool = top.enter_context(tc.tile_pool(name="xt", bufs=1))
        xf_pool = top.enter_context(tc.tile_pool(name="xfull", bufs=1))
        # PSUM pool stack (LIFO dealloc): tp (lives through pass 2) ->
        # spsum (closed early in phase B) -> prep (closed end of pass 1)
        tp_stack = ExitStack()
        tp_pool = tp_stack.enter_context(tc.tile_pool(name="tp", bufs=2, space=PSUM))
        s_stack = ExitStack()
        s_pool = s_stack.enter_context(tc.tile_pool(name="spsum", bufs=1, space=PSUM))
        prep_stack = ExitStack()
        prep_pool = prep_stack.enter_context(
            tc.tile_pool(name="prep", bufs=1, space=PSUM)
        )

        # ------------- const tiles -------------
        identity_f = consts.tile([P, P], F32)
        ident_h = consts.tile([P, P], F16)
        p8 = consts.tile([HEADS, C], F32)
        p8_r = consts.tile([HEADS, C], F32R)
        ones_col = consts.tile([P, 1], F16)
        ones_row = consts.tile([1, P], F32)
        ones_row_h = consts.tile([1, P], F16)
        m1024 = consts.tile([1, P], F16)            # blockdiag mask: -B rank-1
        mneg = consts.tile([1, P], F16)
        p8c = consts.tile([P // DH, P], F16)        # +B rank-4 in-block factors
        p8c2 = consts.tile([P // DH, P], F16)

        # weights (fp16 via gpsimd casting DMA, one per weight)
        wq_h = consts.tile([P, NCHUNK, C], F16)
        wk_h = consts.tile([P, NCHUNK, C], F16)
        wv_h = consts.tile([P, NCHUNK, C], F16)
        wp_h = consts.tile([P, NCHUNK, C], F16)
        wvT = consts.tile([P, NCHUNK, C], F16)      # wvT[p,q,k] = Wv[k, 128q+p]
        wq_s = consts.tile([P, NCHUNK, C], F16)     # Wq * rexp^-2 (norm fork)
        bp_col = consts.tile([P, NCHUNK], F32)      # bp as per-partition column
        resc_p = consts.tile([HEADS, 1], F32)
        resc_r = consts.tile([HEADS, 1], F32R)
        rexp_row = consts.tile([1, C], F32)         # rescale broadcast over blocks
        rexp1i = consts.tile([1, C], F32)
        rexp2i = consts.tile([1, C], F32)
        wbig = [consts.tile([P, C], F16, name=f"wbig{m}") for m in range(NCHUNK)]

        # X (fp16, resident, blocked 16 tokens/partition) and X^T (fp16).
        # xg[g][p, j, :] = x[g*2048 + 16*p + j, :]   (tile (g,j) = tokens
        # {16p+j}); xT[:, k, 128*t + u] = tile t's transpose column u, i.e.
        # token g*2048 + 16*u + j for t = g*16 + j.
        xg = [xf_pool.tile([P, GT, C], F16, name=f"xg{g}") for g in range(NG)]
        x8 = [xf_pool.tile([P, GT, C], F8, name=f"x8{g}") for g in range(NG)]
        xT = xt_pool.tile([P, NCHUNK, N], F16)

        # S accumulator: [S00|S01] at 0:256, S11 at 256:384 -- one PSUM bank,
        # one zero-region so a single start=True covers both.
        s_ps = s_pool.tile([P, 384], F32, space=PSUM)

        # ---------------- pass 1: load X (fp16), S = X^T X, transposes ----------------
        def s_super(st, first=False, last=False):
            # fp8e4 DoubleRow: one matmul contracts 256 tokens (2 tiles).
            # symmetric S: [S00|S01] from lhsT=chunk0; S11 from lhsT=chunk1.
            g, a = divmod(2 * st, GT)
            pair = x8[g][:, ds(a, 2), :]
            nc.tensor.matmul(
                s_ps[:, 0:C], pair[:, :, 0:P], pair,
                start=first, stop=False, perf_mode=mybir.MatmulPerfMode.DoubleRow,
            )
            nc.tensor.matmul(
                s_ps[:, C : C + P], pair[:, :, P:C], pair[:, :, P:C],
                start=False, stop=last, perf_mode=mybir.MatmulPerfMode.DoubleRow,
            )

        def cast_chunk(g, j0, nj, on_dve):
            src_v = xg[g][:, ds(j0, nj), :]
            dst_v = x8[g][:, ds(j0, nj), :]
            if on_dve:
                nc.vector.tensor_copy(dst_v, src_v)
            else:
                nc.scalar.copy(dst_v, src_v)

        emitted_quads = 0

        def dummies(n):
            # dependency-free PE filler: keeps the pstate ramp alive across
            # known stall windows (any PE idle resets the 3us ramp clock)
            for _ in range(n):
                dscr = tp_pool.tile([P, P], F32, space=PSUM, tag="dum", bufs=1)
                nc.tensor.matmul(
                    dscr[:], ones_row_h[:], ones_row_h[:], start=True, stop=True
                )

        def emit_quad():
            # transpose 4 token tiles (both chunks) PE->PSUM, evict to xT
            nonlocal emitted_quads
            if emitted_quads >= NQ:
                return
            q = emitted_quads
            emitted_quads += 1
            tp = tp_pool.tile([P, NCHUNK, QT, P], F16, space=PSUM, tag="tp")
            for j in range(QT):
                t = q * QT + j
                g, a = divmod(t, GT)
                for k in range(NCHUNK):
                    nc.tensor.transpose(
                        tp[:, k, j, :], xg[g][:, a, ts(k, P)], ident_h[:]
                    )
            dst = xT[:, :, ds(q * QT * P, QT * P)].rearrange(
                "p k (j u) -> p k j u", u=P
            )
            if q % 2 == 0:
                nc.vector.tensor_copy(dst, tp[:])
            else:
                nc.scalar.copy(dst, tp[:])

        def x_dma(g, j0, j1):
            nc.gpsimd.dma_start(
                xg[g][:, ds(j0, j1 - j0), :],
                x_dram[ds(g * GT * P, GT * P), :].rearrange(
                    "(p j) c -> p j c", j=GT
                )[:, ds(j0, j1 - j0), :],
            )

        for g in range(NG):
            if g == 0:
                for j0, j1 in ((0, 2), (2, 8), (8, GT)):
                    x_dma(g, j0, j1)
                cast_chunk(0, 0, 2, True)
                cast_chunk(0, 2, 2, False)
                # single activation-table load for the whole kernel
                nc.scalar.add_instruction(
                    mybir.InstLoadActFuncSet(
                        name=nc.get_next_instruction_name(),
                        act_func_set_id=ACT_SET_LN_EXP,
                        ins=[],
                        outs=[],
                    )
                )
                make_identity(nc, identity_f[:])
                nc.scalar.copy(ident_h[:], identity_f[:])
                nc.gpsimd.memset(p8[:], 0.0)
                nc.gpsimd.affine_select(
                    out=p8[:].rearrange("p (b i) -> p b i", i=DH),
                    in_=p8[:].rearrange("p (b i) -> p b i", i=DH),
                    compare_op=ALU.not_equal,
                    fill=1.0,
                    base=0,
                    pattern=[[-1, HEADS], [0, DH]],
                    channel_multiplier=1,
                )
                nc.vector.tensor_copy(p8_r[:], p8[:])
                nc.gpsimd.memset(ones_col[:], 1.0)
                nc.gpsimd.memset(ones_row[:], 1.0)
                nc.vector.tensor_copy(ones_row_h[:], ones_row[:])
                # blockdiag mask factors: -B everywhere (rank 1) + B in-block
                # (rank 4, from the p8 head pattern restricted to one chunk)
                nc.gpsimd.memset(m1024[:], MROW)
                nc.gpsimd.memset(mneg[:], -MCOL)
                nc.vector.tensor_scalar_mul(p8c[:], p8[0 : P // DH, 0:P], MROW)
                nc.vector.tensor_scalar_mul(p8c2[:], p8[0 : P // DH, 0:P], MCOL)
            else:
                x_dma(g, 0, GT)
            if g == 1:
                # weights: casting DMAs (Pool-only) straight to fp16
                for wh, wd in (
                    (wq_h, wq_dram), (wk_h, wk_dram),
                    (wv_h, wv_dram), (wp_h, wp_dram),
                ):
                    nc.gpsimd.dma_start(
                        wh[:], wd.rearrange("(k p) c -> p k c", p=P)
                    )
                nc.sync.dma_start(
                    bp_col[:], bp_dram.rearrange("(k p) -> p k", p=P)
                )
                nc.sync.dma_start(resc_p[:], resc_dram.rearrange("h a b -> h (a b)"))
                nc.vector.tensor_copy(resc_r[:], resc_p[:])

        # PE stream: warmup dummies start the pstate ramp at t~0.5us so the
        # first S matmul already runs at full clock; then S + prep + quads.
        # Casts (fp16->fp8, 2x_2p) run on DVE (3/4) and ACT (1/4).
        dummies(40)
        s_super(0, first=True)
        cast_chunk(0, 4, 4, True)
        s_super(1)
        s_super(2)
        # prep block 1: Wv transposes (fp16, one packed PSUM bank), rexp row
        tpv4 = prep_pool.tile([P, 4, P], F16, space=PSUM, tag="tpv")
        for q in range(NCHUNK):
            for m in range(NCHUNK):
                nc.tensor.transpose(
                    tpv4[:, 2 * q + m, :], wv_h[:, m, ts(q, P)], ident_h[:]
                )
        for q in range(NCHUNK):
            dst = wvT[:, q, :].rearrange("p (m u) -> p m u", u=P)
            if q == 0:
                nc.vector.tensor_copy(dst, tpv4[:, ds(2 * q, 2), :])
            else:
                nc.scalar.copy(dst, tpv4[:, ds(2 * q, 2), :])
        rexp_ps = prep_pool.tile([P, C], F32, space=PSUM, tag="bc")
        nc.tensor.matmul(rexp_ps[0:1, :], resc_r[:], p8_r[:], start=True, stop=True)
        nc.scalar.copy(rexp_row[:], rexp_ps[0:1, :])
        nc.vector.reciprocal(rexp1i[:], rexp_row[:])
        nc.vector.tensor_mul(rexp2i[:], rexp1i[:], rexp1i[:])
        cast_chunk(0, 8, 4, True)
        cast_chunk(0, 12, 4, False)
        for st in range(3, 8):
            s_super(st)
        # prep block 2: rexp^-2 broadcast + scaled Wq (reuses the bc bank)
        r2bc_ps = prep_pool.tile([P, C], F32, space=PSUM, tag="bc")
        nc.tensor.matmul(r2bc_ps[:], ones_row[:], rexp2i[:], start=True, stop=True)
        for k in range(NCHUNK):
            nc.vector.tensor_mul(wq_s[:, k, :], wq_h[:, k, :], r2bc_ps[:])
        prep_stack.close()  # tpv/bc banks free from here
        emit_quad()
        for g in range(1, NG):
            # casts for group g (DVE-heavy), S supertiles, then g-1's quads
            cast_chunk(g, 0, 4, True)
            cast_chunk(g, 4, 4, False)
            cast_chunk(g, 8, 4, True)
            cast_chunk(g, 12, 4, True)
            for st in range(g * GT // 2, (g + 1) * GT // 2):
                s_super(st, last=(st == NT // 2 - 1))
            for _ in range(4):
                emit_quad()
            dummies(6)

        # ---------------- phase B: 256x256 attention math (fp16) ----------------
        # S rows chunk0 = [S00|S01]; chunk1 = [S10|S11] with S10 = S01^T
        #   s_row0 = S[0:128, 0:256], s_row1 = S[128:256, 0:256]
        # lhsT for P* chunk (k, m) = S[k-rows, m-cols] = s_row{k}[:, m*128:]
        with tc.tile_pool(name="bsb0", bufs=1) as bsb0:
            s_row0 = bsb0.tile([P, C], F16)
            s_row1 = bsb0.tile([P, C], F16)
            nc.vector.tensor_copy(s_row0[:, 0:P], s_ps[:, 0:P])
            nc.scalar.copy(s_row0[:, P:C], s_ps[:, P:C])
            nc.vector.tensor_copy(s_row1[:, P:C], s_ps[:, C : C + P])
            with tc.tile_pool(name="preb", bufs=1, space=PSUM) as pre_b:
                s10_ps = pre_b.tile([P, P], F16, space=PSUM, tag="bs16")
                nc.tensor.transpose(s10_ps[:], s_row0[:, P:C], ident_h[:])
                nc.vector.tensor_copy(s_row1[:, 0:P], s10_ps[:])
            s_stack.close()  # S bank free from here on

            emit_quad()
            emit_quad()
            dummies(12)

            srows = [s_row0, s_row1]
            bwork_ctx = ExitStack()
            bwork = bwork_ctx.enter_context(
                tc.tile_pool(name="bwork", bufs=4, space=PSUM)
            )
            bsmall = bwork_ctx.enter_context(
                tc.tile_pool(name="bsmall", bufs=1, space=PSUM)
            )
            bsb = bwork_ctx.enter_context(tc.tile_pool(name="bsb", bufs=1))
            # P1 = S Wq, P2 = S Wk
            p1_ps, p2_ps = [], []
            for dst_list, w_h in ((p1_ps, wq_h), (p2_ps, wk_h)):
                for m in range(NCHUNK):
                    pp = bwork.tile(
                        [P, C], F32, space=PSUM,
                        name=f"pps{len(dst_list)}{m}", tag="bw", bufs=4,
                    )
                    for k in range(NCHUNK):
                        nc.tensor.matmul(
                            pp[:], srows[k][:, ts(m, P)], w_h[:, k, :],
                            start=(k == 0), stop=(k == 1),
                        )
                    dst_list.append(pp)

            # evict P1/P2 to fp16; qp for the nq2 fork (reads PSUM directly)
            p1_sb, p2_sb, qpl = [], [], []
            for m in range(NCHUNK):
                psb = bsb.tile([P, C], F16, name=f"p1sb{m}", tag="p1sb", bufs=2)
                nc.vector.tensor_copy(psb[:, 0:P], p1_ps[m][:, 0:P])
                nc.scalar.copy(psb[:, P:C], p1_ps[m][:, P:C])
                p1_sb.append(psb)
                qp = bsb.tile([P, C], F16, name=f"qp{m}", tag="qp", bufs=2)
                nc.vector.tensor_mul(qp[:], wq_s[:, m, :], p1_ps[m][:])
                qpl.append(qp)
            for m in range(NCHUNK):
                psb = bsb.tile([P, C], F16, name=f"p2sb{m}", tag="p2sb", bufs=2)
                nc.scalar.copy(psb[:, 0:P], p2_ps[m][:, 0:P])
                nc.vector.tensor_copy(psb[:, P:C], p2_ps[m][:, P:C])
                p2_sb.append(psb)

            emit_quad()
            emit_quad()
            dummies(12)

            # G (block-diag chunks only) with the mask matmuls folded in:
            # out-of-block entries get -MROW*MCOL so they vanish in the exp.
            g_ps = []
            for m in range(NCHUNK):
                gg = bwork.tile([P, P], F32, space=PSUM, name=f"gps{m}", tag="bw", bufs=4)
                for k in range(NCHUNK):
                    nc.tensor.matmul(
                        gg[:], wk_h[:, k, ts(m, P)], p1_sb[k][:, ts(m, P)],
                        start=(k == 0), stop=False,
                    )
                nc.tensor.matmul(gg[:], m1024[:], mneg[:], start=False, stop=False)
                nc.tensor.matmul(gg[:], p8c[:], p8c2[:], start=False, stop=True)
                g_ps.append(gg)

            # nq2 fork: colsum(qp) -> rq' = rsqrt(nq2 * rexp^-2) = rq * rescale
            nq2_ps = bsmall.tile([1, C], F32, space=PSUM, tag="bs")
            for k in range(NCHUNK):
                nc.tensor.matmul(
                    nq2_ps[:], ones_col[:], qpl[k][:], start=(k == 0), stop=(k == 1)
                )
            lnq = bsb.tile([1, C], F32)
            nc.scalar.activation(lnq[:], nq2_ps[:], ACTF.Ln)
            rq_h = bsb.tile([1, C], F16)
            nc.scalar.activation(rq_h[:], lnq[:], ACTF.Exp, scale=-0.5)
            csbc_ps = bsmall.tile([P, C], F32, space=PSUM, tag="bs")
            nc.tensor.matmul(csbc_ps[:], ones_row_h[:], rq_h[:], start=True, stop=True)
            csbc_sb = bsb.tile([P, C], F16)
            nc.vector.tensor_copy(csbc_sb[:, 0:P], csbc_ps[:, 0:P])
            nc.scalar.copy(csbc_sb[:, P:C], csbc_ps[:, P:C])

            # nk2 fork: diag(Wk^T P2) via Kgram + identity-masked row-reduce
            nk2 = bsb.tile([P, NCHUNK], F32)
            scraps = [bsb.tile([P, P], F32, name=f"scrap{m}") for m in range(NCHUNK)]
            for m in range(NCHUNK):
                kg = bwork.tile([P, P], F32, space=PSUM, name=f"kg{m}", tag="bw", bufs=4)
                for k in range(NCHUNK):
                    nc.tensor.matmul(
                        kg[:], wk_h[:, k, ts(m, P)], p2_sb[k][:, ts(m, P)],
                        start=(k == 0), stop=(k == 1),
                    )
                nc.vector.scalar_tensor_tensor(
                    out=scraps[m][:],
                    in0=kg[:],
                    scalar=1.0,
                    in1=identity_f[:],
                    op0=ALU.mult,
                    op1=ALU.mult,
                    accum_out=nk2[:, m : m + 1],
                )
            lnk = bsb.tile([P, NCHUNK], F32)
            nc.scalar.activation(lnk[:], nk2[:], ACTF.Ln)
            rk = bsb.tile([P, NCHUNK], F32)
            nc.scalar.activation(rk[:], lnk[:], ACTF.Exp, scale=-0.5)

            emit_quad()
            emit_quad()
            dummies(14)

            # softmax tail + T1 + Wbig
            t1_sb = []
            for m in range(NCHUNK):
                dummies(6)
                tt = bsb.tile([P, P], F16, name=f"t{m}", tag="t", bufs=2)
                nc.vector.tensor_mul(tt[:], g_ps[m][:], csbc_sb[:, ts(m, P)])
                e = bsb.tile([P, P], F16, name=f"e{m}", tag="e", bufs=2)
                den = bsb.tile([P, 1], F32, name=f"den{m}", tag="den", bufs=2)
                nc.scalar.activation(
                    e[:], tt[:], ACTF.Exp, scale=rk[:, m : m + 1], accum_out=den[:]
                )
                rden = bsb.tile([P, 1], F32, name=f"rden{m}", tag="rden", bufs=2)
                nc.vector.reciprocal(rden[:], den[:])
                a_m = bsb.tile([P, P], F16, name=f"a{m}", tag="a", bufs=2)
                nc.vector.tensor_scalar_mul(a_m[:], e[:], rden[:])
                t1p = bwork.tile(
                    [P, C], F32, space=PSUM, name=f"t1ps{m}", tag="bw", bufs=4
                )
                nc.tensor.matmul(t1p[:], a_m[:], wp_h[:, m, :], start=True, stop=True)
                t1s = bsb.tile([P, C], F16, name=f"t1sb{m}", tag="t1sb", bufs=2)
                nc.vector.tensor_copy(t1s[:, 0:P], t1p[:, 0:P])
                nc.scalar.copy(t1s[:, P:C], t1p[:, P:C])
                t1_sb.append(t1s)

            for m in range(NCHUNK):
                wbp = bwork.tile(
                    [P, C], F32, space=PSUM, name=f"wbps{m}", tag="bw", bufs=4
                )
                for q in range(NCHUNK):
                    nc.tensor.matmul(
                        wbp[:], wvT[:, q, ts(m, P)], t1_sb[q][:],
                        start=(q == 0), stop=(q == 1),
                    )
                nc.vector.tensor_copy(wbig[m][:, 0:P], wbp[:, 0:P])
                nc.scalar.copy(wbig[m][:, P:C], wbp[:, P:C])
                dummies(2)
            bwork_ctx.close()

        # ------- pass 2: out^T = Wbig^T xT + bp, 32 pipelined fp16 DMAs -------
        # Output quad oq covers TRUE tokens [oq*512, (oq+1)*512): group
        # g = oq//4, u in [32*(oq%4), +32), all j in [0,16).  The matmul rhs
        # gathers the scattered xT positions; the eviction AP un-permutes
        # (j,u) -> 16u+j so each DMA writes a contiguous token range.
        with tc.tile_pool(name="ops", bufs=5, space=PSUM) as ops, tc.tile_pool(
            name="outb", bufs=4
        ) as outb:
            for oq in range(NQ):
                g, uq = divmod(oq, NQ // NG)
                # keep transposes one group ahead of the output quads
                while emitted_quads < min((g + 2) * (NQ // NG), NQ):
                    emit_quad()
                ob = outb.tile([P, NCHUNK, QT * P], F16, tag="ob")
                for m in range(NCHUNK):
                    o_ps = ops.tile([P, QT * P], F32, space=PSUM, tag="o")
                    for k in range(NCHUNK):
                        rhs = xT[:, k, ds(g * GT * P, GT * P)].rearrange(
                            "p (j u) -> p j u", u=P
                        )[:, :, ds(uq * 32, 32)]
                        nc.tensor.matmul(
                            o_ps[:].rearrange("p (j u) -> p j u", u=32),
                            wbig[k][:, ts(m, P)],
                            rhs,
                            start=(k == 0),
                            stop=(k == 1),
                        )
                    # evict + bias; o_ps columns are (j, u), true token
                    # offset within the quad is 16u + j
                    dst = ob[:, m, :].rearrange("p (u j) -> p u j", j=GT)
                    src = o_ps[:].rearrange("p (j u) -> p u j", u=32)
                    if (oq + m) % 2 == 0:
                        nc.vector.tensor_scalar_add(dst, src, bp_col[:, m : m + 1])
                    else:
                        nc.scalar.activation(
                            dst, src, ACTF.Identity, bias=bp_col[:, m : m + 1]
                        )
                nc.sync.dma_start(outT_v[:, :, ds(oq * QT * P, QT * P)], ob[:])

        tp_stack.close()

    return nc


_NC_CACHE = None


def _get_nc():
    global _NC_CACHE
    if _NC_CACHE is None:
        nc = bacc.Bacc(
            "TRN2",
            target_bir_lowering=False,
            debug=False,
            enable_asserts=False,
            num_devices=N_CORES,
        )
        _build_kernel(nc)
        nc.compile()
        nc.m = get_hw_module(nc.m)
        _NC_CACHE = nc
    return _NC_CACHE


def _make_in_maps(x_in, Wq, Wk, Wv, rescale, Wp, bp):
    x_in = np.ascontiguousarray(np.asarray(x_in, dtype=np.float32))
    maps = []
    for core in range(N_CORES):
        maps.append(
            {
                "x_in": x_in[core].reshape(N, C),
                "Wq": np.asarray(Wq, np.float32),
                "Wk": np.asarray(Wk, np.float32),
                "Wv": np.asarray(Wv, np.float32),
                "rescale": np.asarray(rescale, np.float32),
                "Wp": np.asarray(Wp, np.float32),
                "bp": np.asarray(bp, np.float32),
            }
        )
    return maps


def run_on_hw(inputs: dict, trace: bool = False, tmpdir: str | None = None):
    """Returns (full_output [8,128,128,256] f32, BassKernelResults)."""
    nc = _get_nc()
    in_maps = _make_in_maps(**inputs)
    res = bass_utils.run_bass_kernel_spmd(
        nc, in_maps, core_ids=list(range(N_CORES)), trace=trace, tmpdir=tmpdir
    )
    out = np.stack(
        [
            np.asarray(res.results[c]["out"], dtype=np.float32).T.reshape(H, W, C)
            for c in range(N_CORES)
        ]
    )
    return out, res


def kernel(x_in, Wq, Wk, Wv, rescale, Wp, bp) -> np.ndarray:
    out, _ = run_on_hw(
        dict(x_in=x_in, Wq=Wq, Wk=Wk, Wv=Wv, rescale=rescale, Wp=Wp, bp=bp)
    )
    return out
